# revision 1
# baseline (speedup 1.0000x reference)
"""GCN classifier on 8 TRN2 NeuronCores.

Math (reference):
    h1  = relu(adj @ (X @ W1) + b1)        [N, D]
    h2  = relu(adj @ (h1 @ W2) + b2)       [N, D]
    h3  = relu(h2 @ Wm1 + bm1)             [N, D]
    out = h3 @ Wm2 + bm2                   [N, 1]

Sharding: 1D row partition of adj over 8 cores (2048 rows each). Each core
receives its shard PRE-TRANSPOSED on the host (B_c = adj[rows_c, :].T,
shape [N, P]) so that every on-device matmul contracts over the SBUF
partition axis with operands in natural layout:

    layer 1:  Z1.T = X.T @ B_c            (lhsT = X k-block tiles [128, 64])
              h1.T = relu(W1.T @ Z1.T + b1)
    gather:   AllGather h1.T shards -> full h1.T on every core
    layer 2:  G[kb] = (h1.T slice).T @ W2  (tiny matmul == free transpose)
              Z2.T = G.T @ B_c, h2.T = relu(Z2.T + b2)
    head:     h3.T = relu(Wm1.T @ h2.T + bm1); out.T = Wm2.T @ h3.T + bm2

HBM traffic per core is dominated by streaming B_c twice (2 x 128 MiB).
"""

import numpy as np

N = 16384
D = 64
N_CORES = 8
P = N // N_CORES          # 2048 nodes per core
KB = N // 128             # 128 contraction blocks of 128
T_PER_CORE = P // 128     # 16 local row-blocks per core
IC = 512                  # matmul moving free-dim chunk
NIC = P // IC             # 4 chunks per slab
SLAB_KB = 2               # k-blocks per DMA slab (2 MiB per dma_start)

# fp32 matmuls run at 4 cycles/row on the PE; float32r runs at 1 cycle/row
# for moving dims >= 256 (same bits in SBUF, reduced-precision multiply).
MM_DTYPE_NAME = "float32"

_cache = {}


def _build(mm_dtype_name=MM_DTYPE_NAME, reps=1, use_collective=True,
           slab_kb=SLAB_KB, slab_bufs=3, slab_engines=("sync",)):
    import concourse.bass as bass  # noqa: F401  (registers engines)
    import concourse.mybir as mybir
    import concourse.tile as tile
    from concourse import bacc

    f32 = mybir.dt.float32
    mmdt = getattr(mybir.dt, mm_dtype_name)

    nc = bacc.Bacc("TRN2", target_bir_lowering=False, debug=False,
                   num_devices=N_CORES)

    adjT = nc.dram_tensor("adjT", [N, P], mmdt, kind="ExternalInput")
    xb = nc.dram_tensor("xb", [128, KB, D], mmdt, kind="ExternalInput")
    w1 = nc.dram_tensor("w1", [D, D], mmdt, kind="ExternalInput")
    b1 = nc.dram_tensor("b1", [D, 1], f32, kind="ExternalInput")
    w2 = nc.dram_tensor("w2", [D, D], mmdt, kind="ExternalInput")
    b2 = nc.dram_tensor("b2", [D, 1], f32, kind="ExternalInput")
    wm1 = nc.dram_tensor("wm1", [D, D], mmdt, kind="ExternalInput")
    bm1 = nc.dram_tensor("bm1", [D, 1], f32, kind="ExternalInput")
    wm2 = nc.dram_tensor("wm2", [D, 1], mmdt, kind="ExternalInput")
    bm2 = nc.dram_tensor("bm2", [1, 1], f32, kind="ExternalInput")
    out = nc.dram_tensor("out", [1, P], f32, kind="ExternalOutput")

    # collective bounce buffers (internal DRAM), one pair per rep
    hb_ins = [nc.dram_tensor(f"hb_in{r}", [D, P], mmdt) for r in range(reps)]
    hb_outs = [nc.dram_tensor(f"hb_out{r}", [N_CORES * D, P], mmdt)
               for r in range(reps)]

    n_slabs = KB // slab_kb
    relu = mybir.ActivationFunctionType.Relu
    ident = mybir.ActivationFunctionType.Identity

    with tile.TileContext(nc) as tc:
        with (
            tc.tile_pool(name="bpool", bufs=slab_bufs) as bpool,
            tc.tile_pool(name="big", bufs=1) as big,
            tc.tile_pool(name="wpool", bufs=1) as wpool,
            tc.tile_pool(name="hv", bufs=2) as hv,
            tc.tile_pool(name="gpool", bufs=3) as gpool,
            tc.tile_pool(name="opool", bufs=1) as opool,
            tc.tile_pool(name="pacc", bufs=1, space="PSUM") as pacc,
            tc.tile_pool(name="psmall", bufs=2, space="PSUM") as psmall,
        ):
            # ---- constants ----
            xfull = big.tile([128, KB, D], mmdt, tag="big")
            nc.sync.dma_start(xfull[:], xb[:])
            w1t = wpool.tile([D, D], mmdt, tag="w1")
            nc.gpsimd.dma_start(w1t[:], w1[:])
            w2t = wpool.tile([D, D], mmdt, tag="w2")
            nc.gpsimd.dma_start(w2t[:], w2[:])
            wm1t = wpool.tile([D, D], mmdt, tag="wm1")
            nc.gpsimd.dma_start(wm1t[:], wm1[:])
            wm2t = wpool.tile([D, 1], mmdt, tag="wm2")
            nc.gpsimd.dma_start(wm2t[:], wm2[:])
            b1t = wpool.tile([D, 1], f32, tag="b1")
            nc.gpsimd.dma_start(b1t[:], b1[:])
            b2t = wpool.tile([D, 1], f32, tag="b2")
            nc.gpsimd.dma_start(b2t[:], b2[:])
            bm1t = wpool.tile([D, 1], f32, tag="bm1")
            nc.gpsimd.dma_start(bm1t[:], bm1[:])
            bm2t = wpool.tile([1, 1], f32, tag="bm2")
            nc.gpsimd.dma_start(bm2t[:], bm2[:])

            for _rep in range(reps):
                hb_in = hb_ins[_rep]
                hb_out = hb_outs[_rep]
                # ---- layer 1: Z1.T = X.T @ B  (accumulate over all 128 kb) ----
                pz = pacc.tile([D, P], f32, tag="acc")
                for s in range(n_slabs):
                    slab = bpool.tile([128, slab_kb, P], mmdt, tag="slab")
                    src = adjT[s * slab_kb * 128:(s + 1) * slab_kb * 128, :]
                    seng = getattr(nc, slab_engines[s % len(slab_engines)])
                    seng.dma_start(slab[:], src.rearrange("(n p) i -> p n i", p=128))
                    for j in range(slab_kb):
                        kb = s * slab_kb + j
                        for ic in range(NIC):
                            nc.tensor.matmul(
                                pz[:, ic * IC:(ic + 1) * IC],
                                xfull[:, kb, :],
                                slab[:, j, ic * IC:(ic + 1) * IC],
                                start=(kb == 0),
                                stop=(kb == KB - 1),
                            )

                # ---- h1.T = relu(W1.T @ Z1.T + b1) ----
                z1 = hv.tile([D, P], mmdt, tag="hv")
                nc.vector.tensor_copy(z1[:], pz[:])
                ph = pacc.tile([D, P], f32, tag="acc")
                for ic in range(NIC):
                    nc.tensor.matmul(ph[:, ic * IC:(ic + 1) * IC], w1t[:],
                                     z1[:, ic * IC:(ic + 1) * IC],
                                     start=True, stop=True)
                h1 = hv.tile([D, P], mmdt, tag="hv")
                nc.scalar.activation(h1[:], ph[:], relu, bias=b1t[:])

                # ---- AllGather h1.T ----
                nc.sync.dma_start(hb_in[:], h1[:])
                if use_collective:
                    nc.gpsimd.collective_compute(
                        "AllGather",
                        mybir.AluOpType.bypass,
                        replica_groups=[list(range(N_CORES))],
                        ins=[hb_in.ap().opt()],
                        outs=[hb_out.ap().opt()],
                    )

                # ---- layer 2: G[kb] = (h1.T slice).T @ W2 ; Z2.T = G.T @ B ----
                # gathered h1.T consumed in per-source-core chunks [D, P]
                pz2 = pacc.tile([D, P], f32, tag="acc")
                spc = T_PER_CORE // slab_kb     # slabs per source core
                for c in range(N_CORES):
                    hchunk = hv.tile([D, P], mmdt, tag="hchunk")
                    if use_collective:
                        nc.sync.dma_start(hchunk[:],
                                          hb_out[c * D:(c + 1) * D, :])
                    else:
                        nc.sync.dma_start(hchunk[:], hb_in[:])
                    for s_l in range(spc):
                        s = c * spc + s_l
                        slab = bpool.tile([128, slab_kb, P], mmdt, tag="slab")
                        src = adjT[s * slab_kb * 128:(s + 1) * slab_kb * 128, :]
                        seng = getattr(nc, slab_engines[s % len(slab_engines)])
                        seng.dma_start(slab[:],
                                       src.rearrange("(n p) i -> p n i", p=128))
                        for j in range(slab_kb):
                            kb = s * slab_kb + j
                            t_ = kb % T_PER_CORE
                            pg = psmall.tile([128, D], f32, tag="spg")
                            nc.tensor.matmul(pg[:],
                                             hchunk[:, t_ * 128:(t_ + 1) * 128],
                                             w2t[:], start=True, stop=True)
                            g = gpool.tile([128, D], mmdt, tag="g")
                            nc.vector.tensor_copy(g[:], pg[:])
                            for ic in range(NIC):
                                nc.tensor.matmul(
                                    pz2[:, ic * IC:(ic + 1) * IC],
                                    g[:],
                                    slab[:, j, ic * IC:(ic + 1) * IC],
                                    start=(kb == 0),
                                    stop=(kb == KB - 1),
                                )

                # ---- h2.T = relu(Z2.T + b2) ----
                h2 = hv.tile([D, P], mmdt, tag="hv")
                nc.scalar.activation(h2[:], pz2[:], relu, bias=b2t[:])

                # ---- head ----
                p3 = pacc.tile([D, P], f32, tag="acc")
                for ic in range(NIC):
                    nc.tensor.matmul(p3[:, ic * IC:(ic + 1) * IC], wm1t[:],
                                     h2[:, ic * IC:(ic + 1) * IC],
                                     start=True, stop=True)
                h3 = hv.tile([D, P], mmdt, tag="hv")
                nc.scalar.activation(h3[:], p3[:], relu, bias=bm1t[:])

                outsb = opool.tile([1, P], f32, tag="out")
                for ic in range(NIC):
                    po = psmall.tile([1, IC], f32, tag="spg")
                    nc.tensor.matmul(po[:], wm2t[:],
                                     h3[:, ic * IC:(ic + 1) * IC],
                                     start=True, stop=True)
                    nc.scalar.activation(outsb[:, ic * IC:(ic + 1) * IC], po[:],
                                         ident, bias=bm2t[:])
                nc.sync.dma_start(out[:], outsb[:])

    nc.compile()
    return nc


def _build_hilo(reps=1, use_collective=True, slab_kb=SLAB_KB, slab_bufs=3,
                slab_engines=("sync",), aux_engine="sync"):
    """bf16 hi/lo split-precision build: adj and X arrive as interleaved
    bf16 (hi, lo) pairs; each accumulation does 3 bf16 passes
    (hi*hi + lo*hi + hi*lo), recovering ~2^-17 relative accuracy at
    1 cycle/row PE throughput. Small matmuls stay fp32."""
    import concourse.bass as bass  # noqa: F401
    import concourse.mybir as mybir
    import concourse.tile as tile
    from concourse import bacc

    f32 = mybir.dt.float32
    bf16 = mybir.dt.bfloat16

    nc = bacc.Bacc("TRN2", target_bir_lowering=False, debug=False,
                   num_devices=N_CORES)

    adjT2 = nc.dram_tensor("adjT2", [N, 2 * P], bf16, kind="ExternalInput")
    xb2 = nc.dram_tensor("xb2", [128, KB, 2, D], bf16, kind="ExternalInput")
    w1 = nc.dram_tensor("w1", [D, D], f32, kind="ExternalInput")
    b1 = nc.dram_tensor("b1", [D, 1], f32, kind="ExternalInput")
    w2 = nc.dram_tensor("w2", [D, D], f32, kind="ExternalInput")
    b2 = nc.dram_tensor("b2", [D, 1], f32, kind="ExternalInput")
    wm1 = nc.dram_tensor("wm1", [D, D], f32, kind="ExternalInput")
    bm1 = nc.dram_tensor("bm1", [D, 1], f32, kind="ExternalInput")
    wm2 = nc.dram_tensor("wm2", [D, 1], f32, kind="ExternalInput")
    bm2 = nc.dram_tensor("bm2", [1, 1], f32, kind="ExternalInput")
    out = nc.dram_tensor("out", [1, P], f32, kind="ExternalOutput")

    hb_ins = [nc.dram_tensor(f"hb_in{r}", [D, P], f32) for r in range(reps)]
    hb_outs = [nc.dram_tensor(f"hb_out{r}", [N_CORES * D, P], f32,
                              addr_space="Shared")
               for r in range(reps)]

    n_slabs = KB // slab_kb
    relu = mybir.ActivationFunctionType.Relu
    ident = mybir.ActivationFunctionType.Identity

    with tile.TileContext(nc) as tc:
        with (
            tc.tile_pool(name="bpool", bufs=slab_bufs) as bpool,
            tc.tile_pool(name="big", bufs=1) as big,
            tc.tile_pool(name="wpool", bufs=1) as wpool,
            tc.tile_pool(name="hv", bufs=2) as hv,
            tc.tile_pool(name="gpool", bufs=3) as gpool,
            tc.tile_pool(name="opool", bufs=1) as opool,
            tc.tile_pool(name="pacc", bufs=1, space="PSUM") as pacc,
            tc.tile_pool(name="psmall", bufs=2, space="PSUM") as psmall,
        ):
            aux = getattr(nc, aux_engine)
            xfull = big.tile([128, KB, 2, D], bf16, tag="big")
            aux.dma_start(xfull[:], xb2[:])
            w1t = wpool.tile([D, D], f32, tag="w1")
            nc.gpsimd.dma_start(w1t[:], w1[:])
            w2t = wpool.tile([D, D], f32, tag="w2")
            nc.gpsimd.dma_start(w2t[:], w2[:])
            wm1t = wpool.tile([D, D], f32, tag="wm1")
            nc.gpsimd.dma_start(wm1t[:], wm1[:])
            wm2t = wpool.tile([D, 1], f32, tag="wm2")
            nc.gpsimd.dma_start(wm2t[:], wm2[:])
            b1t = wpool.tile([D, 1], f32, tag="b1")
            nc.gpsimd.dma_start(b1t[:], b1[:])
            b2t = wpool.tile([D, 1], f32, tag="b2")
            nc.gpsimd.dma_start(b2t[:], b2[:])
            bm1t = wpool.tile([D, 1], f32, tag="bm1")
            nc.gpsimd.dma_start(bm1t[:], bm1[:])
            bm2t = wpool.tile([1, 1], f32, tag="bm2")
            nc.gpsimd.dma_start(bm2t[:], bm2[:])

            def big_layer_accum(psum_t, lhs_hi_fn, lhs_lo_fn, slab_of, kb_range):
                """Emit 3-pass hi/lo accumulation for kb in kb_range."""
                for (s, j, kb, slab) in kb_range:
                    combos = (
                        (lhs_hi_fn(kb), slab[:, j, 0, :]),   # hi*hi
                        (lhs_lo_fn(kb), slab[:, j, 0, :]),   # lo*hi
                        (lhs_hi_fn(kb), slab[:, j, 1, :]),   # hi*lo
                    )
                    for pi, (lhsT, rhs) in enumerate(combos):
                        for ic in range(NIC):
                            nc.tensor.matmul(
                                psum_t[:, ic * IC:(ic + 1) * IC],
                                lhsT,
                                rhs[:, ic * IC:(ic + 1) * IC],
                                start=(kb == 0 and pi == 0),
                                stop=(kb == KB - 1 and pi == 2),
                            )

            for _rep in range(reps):
                hb_in = hb_ins[_rep]
                hb_out = hb_outs[_rep]

                # ---- layer 1 ----
                pz = pacc.tile([D, P], f32, tag="acc")
                for s in range(n_slabs):
                    slab = bpool.tile([128, slab_kb, 2, P], bf16, tag="slab")
                    src = adjT2[s * slab_kb * 128:(s + 1) * slab_kb * 128, :]
                    seng = getattr(nc, slab_engines[s % len(slab_engines)])
                    seng.dma_start(slab[:],
                                   src.rearrange("(n p) (h i) -> p n h i",
                                                 p=128, h=2))
                    big_layer_accum(
                        pz,
                        lambda kb: xfull[:, kb, 0, :],
                        lambda kb: xfull[:, kb, 1, :],
                        None,
                        [(s, j, s * slab_kb + j, slab) for j in range(slab_kb)],
                    )

                z1 = hv.tile([D, P], f32, tag="hv")
                nc.vector.tensor_copy(z1[:], pz[:])
                ph = pacc.tile([D, P], f32, tag="acc")
                for ic in range(NIC):
                    nc.tensor.matmul(ph[:, ic * IC:(ic + 1) * IC], w1t[:],
                                     z1[:, ic * IC:(ic + 1) * IC],
                                     start=True, stop=True)
                h1 = hv.tile([D, P], f32, tag="hv")
                nc.scalar.activation(h1[:], ph[:], relu, bias=b1t[:])

                # ---- AllGather h1.T ----
                aux.dma_start(hb_in[:], h1[:])
                if use_collective:
                    nc.gpsimd.collective_compute(
                        "AllGather",
                        mybir.AluOpType.bypass,
                        replica_groups=[list(range(N_CORES))],
                        ins=[hb_in.ap().opt()],
                        outs=[hb_out.ap().opt()],
                    )

                # ---- layer 2 ----
                pz2 = pacc.tile([D, P], f32, tag="acc")
                spc = T_PER_CORE // slab_kb
                for c in range(N_CORES):
                    hchunk = hv.tile([D, P], f32, tag="hchunk")
                    if use_collective:
                        aux.dma_start(hchunk[:],
                                      hb_out[c * D:(c + 1) * D, :])
                    else:
                        aux.dma_start(hchunk[:], hb_in[:])
                    for s_l in range(spc):
                        s = c * spc + s_l
                        slab = bpool.tile([128, slab_kb, 2, P], bf16, tag="slab")
                        src = adjT2[s * slab_kb * 128:(s + 1) * slab_kb * 128, :]
                        seng = getattr(nc, slab_engines[s % len(slab_engines)])
                        seng.dma_start(slab[:],
                                       src.rearrange("(n p) (h i) -> p n h i",
                                                     p=128, h=2))
                        for j in range(slab_kb):
                            kb = s * slab_kb + j
                            t_ = kb % T_PER_CORE
                            pg = psmall.tile([128, D], f32, tag="spg")
                            nc.tensor.matmul(pg[:],
                                             hchunk[:, t_ * 128:(t_ + 1) * 128],
                                             w2t[:], start=True, stop=True)
                            g32 = gpool.tile([128, D], f32, tag="g32")
                            nc.vector.tensor_copy(g32[:], pg[:])
                            ghi = gpool.tile([128, D], bf16, tag="ghi")
                            nc.vector.tensor_copy(ghi[:], g32[:])
                            glo = gpool.tile([128, D], bf16, tag="glo")
                            nc.vector.tensor_sub(glo[:], g32[:], ghi[:])
                            combos = ((ghi, slab[:, j, 0, :]),
                                      (glo, slab[:, j, 0, :]),
                                      (ghi, slab[:, j, 1, :]))
                            for pi, (lhsT, rhs) in enumerate(combos):
                                for ic in range(NIC):
                                    nc.tensor.matmul(
                                        pz2[:, ic * IC:(ic + 1) * IC],
                                        lhsT[:],
                                        rhs[:, ic * IC:(ic + 1) * IC],
                                        start=(kb == 0 and pi == 0),
                                        stop=(kb == KB - 1 and pi == 2),
                                    )

                h2 = hv.tile([D, P], f32, tag="hv")
                nc.scalar.activation(h2[:], pz2[:], relu, bias=b2t[:])

                p3 = pacc.tile([D, P], f32, tag="acc")
                for ic in range(NIC):
                    nc.tensor.matmul(p3[:, ic * IC:(ic + 1) * IC], wm1t[:],
                                     h2[:, ic * IC:(ic + 1) * IC],
                                     start=True, stop=True)
                h3 = hv.tile([D, P], f32, tag="hv")
                nc.scalar.activation(h3[:], p3[:], relu, bias=bm1t[:])

                outsb = opool.tile([1, P], f32, tag="out")
                for ic in range(NIC):
                    po = psmall.tile([1, IC], f32, tag="spg")
                    nc.tensor.matmul(po[:], wm2t[:],
                                     h3[:, ic * IC:(ic + 1) * IC],
                                     start=True, stop=True)
                    nc.scalar.activation(outsb[:, ic * IC:(ic + 1) * IC], po[:],
                                         ident, bias=bm2t[:])
                nc.sync.dma_start(out[:], outsb[:])

    nc.compile()
    return nc


def _hilo_split(a):
    import ml_dtypes
    hi = a.astype(ml_dtypes.bfloat16)
    lo = (a - hi.astype(np.float32)).astype(ml_dtypes.bfloat16)
    return hi, lo


def _hilo_shard(adj, c):
    """adjT2 for core c: [N, 2P] bf16, rows k = [hi(adj[rows_c, k]) | lo]."""
    import ml_dtypes
    block = np.ascontiguousarray(adj[c * P:(c + 1) * P, :], dtype=np.float32)
    hi = block.astype(ml_dtypes.bfloat16)          # [P, N]
    lo = (block - hi.astype(np.float32)).astype(ml_dtypes.bfloat16)
    a2 = np.empty((N, 2 * P), dtype=ml_dtypes.bfloat16)
    a2u, hiu, lou = a2.view(np.uint16), hi.view(np.uint16), lo.view(np.uint16)
    step = 2048
    for k0 in range(0, N, step):
        a2u[k0:k0 + step, :P] = hiu[:, k0:k0 + step].T
        a2u[k0:k0 + step, P:] = lou[:, k0:k0 + step].T
    return a2


def _prep_inputs_hilo(adj, features, W1, b1, W2, b2, Wm1, bm1, Wm2, bm2):
    from concurrent.futures import ThreadPoolExecutor

    x = np.ascontiguousarray(features, dtype=np.float32)
    xb = np.ascontiguousarray(x.reshape(KB, 128, D).transpose(1, 0, 2))
    xhi, xlo = _hilo_split(xb)
    xb2 = np.ascontiguousarray(
        np.stack([xhi, xlo], axis=2))          # [128, KB, 2, D]
    common = {
        "xb2": xb2,
        "w1": np.ascontiguousarray(W1, dtype=np.float32),
        "b1": np.ascontiguousarray(b1, dtype=np.float32).reshape(D, 1),
        "w2": np.ascontiguousarray(W2, dtype=np.float32),
        "b2": np.ascontiguousarray(b2, dtype=np.float32).reshape(D, 1),
        "wm1": np.ascontiguousarray(Wm1, dtype=np.float32),
        "bm1": np.ascontiguousarray(bm1, dtype=np.float32).reshape(D, 1),
        "wm2": np.ascontiguousarray(Wm2, dtype=np.float32).reshape(D, 1),
        "bm2": np.ascontiguousarray(bm2, dtype=np.float32).reshape(1, 1),
    }
    with ThreadPoolExecutor(max_workers=8) as ex:
        shards = list(ex.map(lambda c: _hilo_shard(adj, c), range(N_CORES)))
    return [dict(common, adjT2=shards[c]) for c in range(N_CORES)]


def _shard_adj(adj):
    """Per-core transposed shards B_c = adj[rows_c, :].T, contiguous."""
    shards = []
    for c in range(N_CORES):
        block = adj[c * P:(c + 1) * P, :]              # [P, N]
        bt = np.empty((N, P), dtype=np.float32)
        # blocked transpose: column-chunk of the source at a time
        step = 1024
        for k0 in range(0, N, step):
            bt[k0:k0 + step, :] = block[:, k0:k0 + step].T
        shards.append(bt)
    return shards


def _prep_inputs(adj, features, W1, b1, W2, b2, Wm1, bm1, Wm2, bm2):
    adj = np.ascontiguousarray(adj, dtype=np.float32)
    x = np.ascontiguousarray(features, dtype=np.float32)
    # xb[p, kb, d] = X[kb*128 + p, d]
    xb = np.ascontiguousarray(x.reshape(KB, 128, D).transpose(1, 0, 2))
    shards = _shard_adj(adj)
    common = {
        "xb": xb,
        "w1": np.ascontiguousarray(W1, dtype=np.float32),
        "b1": np.ascontiguousarray(b1, dtype=np.float32).reshape(D, 1),
        "w2": np.ascontiguousarray(W2, dtype=np.float32),
        "b2": np.ascontiguousarray(b2, dtype=np.float32).reshape(D, 1),
        "wm1": np.ascontiguousarray(Wm1, dtype=np.float32),
        "bm1": np.ascontiguousarray(bm1, dtype=np.float32).reshape(D, 1),
        "wm2": np.ascontiguousarray(Wm2, dtype=np.float32).reshape(D, 1),
        "bm2": np.ascontiguousarray(bm2, dtype=np.float32).reshape(1, 1),
    }
    return [dict(common, adjT=shards[c]) for c in range(N_CORES)]


MODE = "hilo"          # "hilo" (bf16 split, 3-pass) or a mm dtype name
HILO_SLAB_KB = 1
HILO_SLAB_BUFS = 10


def _build_default():
    if MODE == "hilo":
        return _build_hilo(slab_kb=HILO_SLAB_KB, slab_bufs=HILO_SLAB_BUFS)
    return _build(mm_dtype_name=MODE)


def _run(in_maps, trace=False, **kw):
    from concourse.bass_utils import run_bass_kernel_spmd

    if "nc" not in _cache:
        _cache["nc"] = _build_default()
    res = run_bass_kernel_spmd(_cache["nc"], in_maps,
                               core_ids=list(range(N_CORES)),
                               trace=trace, **kw)
    full = np.concatenate([r["out"][0] for r in res.results])[:, None]
    return full.astype(np.float32), res


def kernel(adj, features, W1, b1, W2, b2, Wm1, bm1, Wm2, bm2):
    adj = np.asarray(adj)
    features = np.asarray(features)
    W1, b1, W2, b2 = map(np.asarray, (W1, b1, W2, b2))
    Wm1, bm1, Wm2, bm2 = map(np.asarray, (Wm1, bm1, Wm2, bm2))
    prep = _prep_inputs_hilo if MODE == "hilo" else _prep_inputs
    in_maps = prep(adj, features, W1, b1, W2, b2, Wm1, bm1, Wm2, bm2)
    try:
        out, _ = _run(in_maps)
    except Exception:
        # transient NRT device wedge: wait for the terminal to reset, retry
        import time as _time
        _time.sleep(75)
        out, _ = _run(in_maps)
    return out



# revision 2
# speedup vs baseline: 1.3440x; 1.3440x over previous
"""GCN classifier on 8 TRN2 NeuronCores — uint8-quantized adjacency.

Math (reference):
    h1  = relu(adj @ (X @ W1) + b1)        [N, D]
    h2  = relu(adj @ (h1 @ W2) + b2)       [N, D]
    h3  = relu(h2 @ Wm1 + bm1)             [N, D]
    out = h3 @ Wm2 + bm2                   [N, 1]

Key idea: the adjacency (the only big tensor; read twice) is quantized on
the host to uint8 (q = rint(a*255/amax), error ~0.2%) and shipped to HBM at
1 byte/element — half the bf16 roofline, a quarter of fp32.  The PE cannot
consume uint8 (BIR verifier allows float dtypes only), so each slab is
unpacked on-device:

    DMA   adjT slab as raw bytes (uint16-viewed pairs)     1 B/elem of HBM
    DVE   dual-op (v & 0xFF) | 0x6400 -> fp16 bits of (1024 + q_even)
    DVE   dual-op (v >> 8)  | 0x6400 -> fp16 bits of (1024 + q_odd)
    PE    fp16 matmuls on the bitcast tiles, 2x column-tiled over kb parity

The +1024 magic-number offset is exact (mantissa ulp at 1024 is 1) and is
subtracted via the bias: z = s*(acc - 1024*colsum(lhs)) + b.  colsum(XW1)
is host-computed; colsum(g) is reduced on device with a ones-matmul.

Sharding: 1D row partition of adj (2048 rows/core), each shard
pre-transposed on host to B_c = adj[rows_c, :].T stored [N, P].  All
matmuls contract over the SBUF partition axis:

    layer 1:  Z1.T = (XW1).T @ B_c    (lhsT = host-computed XW1 k-blocks)
    gather:   per-core g = (h1 @ W2) blocks (PE-transposed), AllGather fp16
    layer 2:  Z2.T = G.T @ B_c
    head:     h3.T = relu(Wm1.T @ h2.T + bm1); out.T = Wm2.T @ h3.T + bm2

Layout bookkeeping: byte-pairing along P puts even local nodes in the low
byte, odd in the high byte, so outputs come out column-permuted
[evens | odds]; the contraction (k) rows of B_c and XW1 are host-permuted
per 128-block to [evens | odds] so layer-2's gathered G blocks line up.
The host unpermutes the final [1, P] outputs.
"""

import numpy as np

N = 16384
D = 64
N_CORES = 8
P = N // N_CORES          # 2048 local nodes per core
KB = N // 128             # 128 contraction blocks
T_PER_CORE = P // 128     # 16 local row-blocks per core
SLAB_KB = 4               # k-blocks per DMA slab (1 MiB per dma_start)
SLAB_BUFS = 8
EO_BUFS = 8               # ev/od u16 tiles in flight (2 per slab)

_cache = {}


def _build(reps=1, use_collective=True, slab_kb=SLAB_KB, slab_bufs=SLAB_BUFS,
           eo_bufs=EO_BUFS):
    import concourse.bass as bass  # noqa: F401
    import concourse.mybir as mybir
    import concourse.tile as tile
    from concourse import bacc

    f32 = mybir.dt.float32
    fp16 = mybir.dt.float16
    u16 = mybir.dt.uint16

    AND = mybir.AluOpType.bitwise_and
    SHR = mybir.AluOpType.logical_shift_right
    OR = mybir.AluOpType.bitwise_or
    MUL = mybir.AluOpType.mult
    ADD = mybir.AluOpType.add
    relu = mybir.ActivationFunctionType.Relu
    ident = mybir.ActivationFunctionType.Identity
    copy = mybir.ActivationFunctionType.Copy

    nc = bacc.Bacc("TRN2", target_bir_lowering=False, debug=False,
                   num_devices=N_CORES)

    adjq = nc.dram_tensor("adjq", [N, P // 2], u16, kind="ExternalInput")
    yb = nc.dram_tensor("yb", [128, KB, D], fp16, kind="ExternalInput")
    w2 = nc.dram_tensor("w2", [D, D], fp16, kind="ExternalInput")
    wm1 = nc.dram_tensor("wm1", [D, D], fp16, kind="ExternalInput")
    wm2 = nc.dram_tensor("wm2", [D, 1], fp16, kind="ExternalInput")
    b1 = nc.dram_tensor("b1", [D, 1], f32, kind="ExternalInput")
    b2 = nc.dram_tensor("b2", [D, 1], f32, kind="ExternalInput")
    bm1 = nc.dram_tensor("bm1", [D, 1], f32, kind="ExternalInput")
    bm2 = nc.dram_tensor("bm2", [1, 1], f32, kind="ExternalInput")
    sc = nc.dram_tensor("sc", [D, 1], f32, kind="ExternalInput")  # amax/255
    out = nc.dram_tensor("out", [1, P], f32, kind="ExternalOutput")

    hb_ins = [nc.dram_tensor(f"hb_in{r}", [128, T_PER_CORE * D], fp16)
              for r in range(reps)]
    hb_outs = [nc.dram_tensor(f"hb_out{r}", [N_CORES * 128, T_PER_CORE * D],
                              fp16, addr_space="Shared")
               for r in range(reps)]

    n_slabs = KB // slab_kb
    spc = T_PER_CORE // slab_kb          # slabs per source core (layer 2)
    HP = P // 2                          # 1024

    with tile.TileContext(nc) as tc:
        with (
            tc.tile_pool(name="bpool", bufs=slab_bufs) as bpool,
            tc.tile_pool(name="eo", bufs=eo_bufs) as eo,
            tc.tile_pool(name="wq", bufs=1) as wq,      # yt / g_all (shared)
            tc.tile_pool(name="wpool", bufs=1) as wpool,
            tc.tile_pool(name="hv", bufs=1) as hv,
            tc.tile_pool(name="cv", bufs=1) as cv,
            tc.tile_pool(name="gpool", bufs=2) as gpool,
            tc.tile_pool(name="opool", bufs=1) as opool,
            tc.tile_pool(name="pacc", bufs=1, space="PSUM") as pacc,
            tc.tile_pool(name="psmall", bufs=1, space="PSUM") as psmall,
        ):
            # ---- constants ----
            w2t = wpool.tile([D, D], fp16, tag="w2")
            nc.gpsimd.dma_start(w2t[:], w2[:])
            wm1t = wpool.tile([D, D], fp16, tag="wm1")
            nc.gpsimd.dma_start(wm1t[:], wm1[:])
            wm2t = wpool.tile([D, 1], fp16, tag="wm2")
            nc.gpsimd.dma_start(wm2t[:], wm2[:])
            b1t = wpool.tile([D, 1], f32, tag="b1")
            nc.gpsimd.dma_start(b1t[:], b1[:])
            b2t = wpool.tile([D, 1], f32, tag="b2")
            nc.gpsimd.dma_start(b2t[:], b2[:])
            bm1t = wpool.tile([D, 1], f32, tag="bm1")
            nc.gpsimd.dma_start(bm1t[:], bm1[:])
            bm2t = wpool.tile([1, 1], f32, tag="bm2")
            nc.gpsimd.dma_start(bm2t[:], bm2[:])
            sct = wpool.tile([D, 1], f32, tag="sc")
            nc.gpsimd.dma_start(sct[:], sc[:])
            ones128 = wpool.tile([128, 1], fp16, tag="ones128")
            nc.vector.memset(ones128[:], 1.0)
            onef32 = wpool.tile([1, 1], f32, tag="onef32")
            nc.vector.memset(onef32[:], 1.0)

            def big_layer(lhsT_of, psum_t, kb0_slab):
                """Stream adj slabs; unpack; accumulate into psum_t
                [128, P] (even kb -> partitions 0:64, odd -> 64:128)."""
                for s in range(n_slabs):
                    slab = bpool.tile([128, slab_kb, HP], u16, tag="slab")
                    src = adjq[s * slab_kb * 128:(s + 1) * slab_kb * 128, :]
                    nc.sync.dma_start(
                        slab[:], src.rearrange("(p n) i -> p n i", p=128))
                    ev = eo.tile([128, slab_kb, HP], u16, tag="eo")
                    nc.vector.tensor_scalar(ev[:], slab[:], 0x00FF, 0x6400,
                                            AND, OR)
                    od = eo.tile([128, slab_kb, HP], u16, tag="eo")
                    nc.vector.tensor_scalar(od[:], slab[:], 8, 0x6400,
                                            SHR, OR)
                    for j in range(slab_kb):
                        kb = s * slab_kb + j
                        half = 64 * (kb % 2)
                        tp = (0, half)
                        first = kb < 2
                        last = kb >= KB - 2
                        for ic in range(2):
                            nc.tensor.matmul(
                                psum_t[half:half + D,
                                       ic * 512:(ic + 1) * 512],
                                lhsT_of(kb),
                                ev[:, j, ic * 512:(ic + 1) * 512].bitcast(
                                    fp16),
                                start=first, stop=last, tile_position=tp)
                        for ic in range(2):
                            nc.tensor.matmul(
                                psum_t[half:half + D,
                                       HP + ic * 512:HP + (ic + 1) * 512],
                                lhsT_of(kb),
                                od[:, j, ic * 512:(ic + 1) * 512].bitcast(
                                    fp16),
                                start=first, stop=last, tile_position=tp)

            def combine_relu(psum_t, bias_t, htag):
                """h.T = relu(sc * (psum[0:64] + psum[64:128]) + bias)."""
                cmb = cv.tile([128, P], f32, tag="cmb")
                nc.scalar.activation(cmb[64:128, :], psum_t[64:128, :], copy)
                zlo = cv.tile([D, P], f32, tag="zlo")
                nc.scalar.dma_start(zlo[:], cmb[64:128, :])
                zsum = cv.tile([D, P], f32, tag="zsum")
                nc.vector.tensor_tensor(
                    zsum[:], zlo[:], psum_t[0:D, :], mybir.AluOpType.add)
                ht = hv.tile([D, P], fp16, tag=htag)
                nc.scalar.activation(ht[:], zsum[:], relu, bias=bias_t[:],
                                     scale=sct[:])
                return ht

            for _rep in range(reps):
                hb_in = hb_ins[_rep]
                hb_out = hb_outs[_rep]

                yt = wq.tile([128, KB, D], fp16, tag="wq")
                nc.sync.dma_start(yt[:], yb[:])

                # ---- layer 1 ----
                pz = pacc.tile([128, P], f32, tag="acc")
                big_layer(lambda kb: yt[:, kb, :], pz, 0)
                h1 = combine_relu(pz, b1t, "h1")

                # ---- local g blocks: g = h1 @ W2, PE-transposed ----
                gl = gpool.tile([128, T_PER_CORE, D], fp16, tag="gl")
                for t in range(T_PER_CORE):
                    pg = psmall.tile([128, D], f32, tag="pg")
                    nc.tensor.matmul(pg[0:D, :],
                                     h1[:, t * D:(t + 1) * D], w2t[:],
                                     start=True, stop=True)
                    nc.tensor.matmul(pg[D:128, :],
                                     h1[:, HP + t * D:HP + (t + 1) * D],
                                     w2t[:], start=True, stop=True,
                                     tile_position=(0, 64))
                    nc.scalar.activation(gl[:, t, :], pg[:], copy)
                nc.scalar.dma_start(hb_in[:], gl[:])

                # ---- AllGather g ----
                if use_collective:
                    nc.gpsimd.collective_compute(
                        "AllGather",
                        mybir.AluOpType.bypass,
                        replica_groups=[list(range(N_CORES))],
                        ins=[hb_in.ap().opt()],
                        outs=[hb_out.ap().opt()],
                    )
                g_all = wq.tile([128, KB, D], fp16, tag="wq")
                for src in range(N_CORES):
                    if use_collective:
                        nc.scalar.dma_start(
                            g_all[:, src * T_PER_CORE:(src + 1) * T_PER_CORE,
                                  :],
                            hb_out[src * 128:(src + 1) * 128, :].rearrange(
                                "p (t d) -> p t d", d=D))
                    else:
                        nc.scalar.dma_start(
                            g_all[:, src * T_PER_CORE:(src + 1) * T_PER_CORE,
                                  :],
                            hb_in[:].rearrange("p (t d) -> p t d", d=D))

                # ---- bias2' = b2 - 1024*s1*colsum(g) ----
                pcs = psmall.tile([1, D], f32, tag="pcs")
                for kb in range(KB):
                    nc.tensor.matmul(pcs[:], ones128[:], g_all[:, kb, :],
                                     start=(kb == 0), stop=(kb == KB - 1))
                pcs_sb = cv.tile([1, D], f32, tag="pcs_sb")
                nc.scalar.activation(pcs_sb[:], pcs[:], copy)
                pct = psmall.tile([D, 1], f32, tag="pct")
                nc.tensor.matmul(pct[:], pcs_sb[:], onef32[:],
                                 start=True, stop=True)
                t1 = cv.tile([D, 1], f32, tag="t1")
                nc.vector.tensor_tensor(t1[:], pct[:], sct[:], MUL)
                b2v = cv.tile([D, 1], f32, tag="b2v")
                nc.vector.tensor_scalar(t1[:], t1[:], -1024.0, None, MUL)
                nc.vector.tensor_tensor(b2v[:], b2t[:], t1[:], ADD)

                # ---- layer 2 ----
                pz2 = pacc.tile([128, P], f32, tag="acc")
                big_layer(lambda kb: g_all[:, kb, :], pz2, 0)
                h2 = combine_relu(pz2, b2v, "h2")

                # ---- head ----
                p3 = pacc.tile([D, P], f32, tag="acc")
                for ic in range(4):
                    nc.tensor.matmul(p3[:, ic * 512:(ic + 1) * 512], wm1t[:],
                                     h2[:, ic * 512:(ic + 1) * 512],
                                     start=True, stop=True)
                h3 = hv.tile([D, P], fp16, tag="h3")
                nc.scalar.activation(h3[:], p3[:], relu, bias=bm1t[:])

                outsb = opool.tile([1, P], f32, tag="out")
                for ic in range(4):
                    po = psmall.tile([1, 512], f32, tag="po")
                    nc.tensor.matmul(po[:], wm2t[:],
                                     h3[:, ic * 512:(ic + 1) * 512],
                                     start=True, stop=True)
                    nc.scalar.activation(outsb[:, ic * 512:(ic + 1) * 512],
                                         po[:], ident, bias=bm2t[:])
                nc.sync.dma_start(out[:], outsb[:])

    nc.compile()
    return nc


def _perm_k():
    """Per-128-block [evens | odds] permutation of row indices."""
    i = np.arange(N)
    b, r = i // 128, i % 128
    return b * 128 + np.where(r < 64, 2 * r, 2 * (r - 64) + 1)


_SIGMA = None


def _sigma():
    """Output column m -> local node index."""
    global _SIGMA
    if _SIGMA is None:
        m = np.arange(P)
        _SIGMA = np.where(m < P // 2, 2 * m, 2 * (m - P // 2) + 1)
    return _SIGMA


def _shard_adjq(adj, scale_inv, pk, c):
    """Quantize + transpose + k-permute + slab-swizzle one core's shard.

    Storage row s*G + p*slab_kb + j holds logical (permuted) row
    s*G + j*128 + p, so each partition's slab read is one contiguous
    slab_kb*P-byte run (G = slab_kb*128 rows per slab)."""
    block = adj[c * P:(c + 1) * P, :]                  # [P, N] f32
    q = np.empty((N, P), dtype=np.uint8)               # transposed
    step = 2048
    for k0 in range(0, N, step):
        sub = block[:, k0:k0 + step].T * scale_inv     # [step, P]
        q[k0:k0 + step, :] = np.rint(sub).astype(np.uint8)
    r = np.arange(N)
    G = SLAB_KB * 128
    s_, rem = r // G, r % G
    p_, j_ = rem // SLAB_KB, rem % SLAB_KB
    src = pk[s_ * G + j_ * 128 + p_]
    qp = q[src, :]
    return np.ascontiguousarray(qp).view(np.uint16)    # [N, P//2]


def _prep_inputs(adj, features, W1, b1, W2, b2, Wm1, bm1, Wm2, bm2):
    from concurrent.futures import ThreadPoolExecutor

    adj = np.asarray(adj, dtype=np.float32)
    amax = float(adj.max())
    scale_inv = 255.0 / amax
    pk = _perm_k()

    y = np.asarray(features, np.float32) @ np.asarray(W1, np.float32)
    y = y[pk, :]                                       # [N, 64] permuted
    ybv = np.ascontiguousarray(
        y.reshape(KB, 128, D).transpose(1, 0, 2)).astype(np.float16)
    csy = ybv.astype(np.float64).sum(axis=(0, 1))      # colsum of fp16 y
    b1eff = (np.asarray(b1, np.float64)
             - (amax / 255.0) * 1024.0 * csy).astype(np.float32)

    common = {
        "yb": ybv,
        "w2": np.asarray(W2, np.float16),
        "wm1": np.asarray(Wm1, np.float16),
        "wm2": np.asarray(Wm2, np.float16).reshape(D, 1),
        "b1": b1eff.reshape(D, 1),
        "b2": np.asarray(b2, np.float32).reshape(D, 1),
        "bm1": np.asarray(bm1, np.float32).reshape(D, 1),
        "bm2": np.asarray(bm2, np.float32).reshape(1, 1),
        "sc": np.full((D, 1), amax / 255.0, np.float32),
    }
    with ThreadPoolExecutor(max_workers=8) as ex:
        shards = list(ex.map(
            lambda c: _shard_adjq(adj, scale_inv, pk, c), range(N_CORES)))
    return [dict(common, adjq=shards[c]) for c in range(N_CORES)]


def _run(in_maps, **kw):
    from concourse.bass_utils import run_bass_kernel_spmd

    if "nc" not in _cache:
        _cache["nc"] = _build()
    res = run_bass_kernel_spmd(_cache["nc"], in_maps,
                               core_ids=list(range(N_CORES)), **kw)
    sig = _sigma()
    full = np.empty((N,), np.float32)
    for c in range(N_CORES):
        full[c * P + sig] = res.results[c]["out"][0]
    return full[:, None], res


def kernel(adj, features, W1, b1, W2, b2, Wm1, bm1, Wm2, bm2):
    in_maps = _prep_inputs(adj, features, W1, b1, W2, b2, Wm1, bm1, Wm2, bm2)
    try:
        out, _ = _run(in_maps)
    except Exception:
        import time as _time
        _time.sleep(75)
        out, _ = _run(in_maps)
    return out
